# revision 1
# baseline (speedup 1.0000x reference)
import os
import sys

import numpy as np

for _p in ("/opt/trn_rl_repo",):
    if _p not in sys.path and os.path.isdir(_p):
        sys.path.append(_p)

N = 1500
A = 64
STD = 0.3
PERSON_IDX = 2
INV2S2 = 1.0 / (2.0 * STD * STD)
SCALE = 2.0 * INV2S2

P = 128
NO = 1536            # padded objects (8 cores x 192)
NCORES = 8
OPC = NO // NCORES   # 192 objects per core
NOC = 256            # per-core object padding (2 tiles of 128)
NT = NOC // P        # 2 object tiles per core
KMAX = 24            # persons per device batch
GP = 8               # persons per matmul group
NG = KMAX // GP      # 4 groups
KK = 6 * GP + 1      # 49 contraction rows
NF = GP * A          # 512 free columns (person-in-group x action)

NEG = -1.0e9
TCLAMP = 16.0        # |t| clamp; any clamped pair has exp(-inv2s2*(16-2)^2) = 0
LNFLOOR = -20000.0   # floor for lnobj/SCALE row (fp16-safe, still exp -> 0)


def _mode():
    return os.environ.get("KERNEL_MM", "fp16hl")


def _hilo(a):
    hi = a.astype(np.float16)
    lo = (a - hi.astype(np.float32)).astype(np.float16)
    return hi, lo


def _obj_arrays(bbox, scores):
    best = scores.max(axis=1)
    idx = scores.argmax(axis=1)
    person = idx == PERSON_IDX
    obj = np.where(person, 0.0, best).astype(np.float32)

    w = bbox[:, 2] - bbox[:, 0]
    h = bbox[:, 3] - bbox[:, 1]
    cx = bbox[:, 0] + 0.5 * w
    cy = bbox[:, 1] + 0.5 * h

    cx_p = np.zeros(NO, np.float32); cx_p[:N] = cx
    cy_p = np.zeros(NO, np.float32); cy_p[:N] = cy
    lw_p = np.zeros(NO, np.float32); lw_p[:N] = np.log(w)
    lh_p = np.zeros(NO, np.float32); lh_p[:N] = np.log(h)
    lnobj_p = np.full(NO, NEG, np.float32)
    pos = obj > 0
    lnobj_p[:N] = np.where(pos, np.log(np.maximum(obj, 1e-38)), NEG)
    return person, best, w, h, cx, cy, cx_p, cy_p, lw_p, lh_p, lnobj_p


def _host_prep(hidx, best, w, h, cx, cy, obj_arr, target_mean, action_logits):
    """Build in_maps for one batch of <=KMAX persons (object-axis sharding)."""
    cx_p, cy_p, lw_p, lh_p, lnobj_p = obj_arr
    k = len(hidx)

    invw = np.ones(KMAX, np.float32); invw[:k] = 1.0 / w[hidx]
    invh = np.ones(KMAX, np.float32); invh[:k] = 1.0 / h[hidx]
    cxh = np.zeros(KMAX, np.float32); cxh[:k] = cx[hidx]
    cyh = np.zeros(KMAX, np.float32); cyh[:k] = cy[hidx]
    lwh = np.zeros(KMAX, np.float32); lwh[:k] = np.log(w[hidx])
    lhh = np.zeros(KMAX, np.float32); lhh[:k] = np.log(h[hidx])
    mu = np.zeros((KMAX, A, 4), np.float32); mu[:k] = target_mean[hidx]
    m2 = (mu * mu).sum(axis=-1)                               # [KMAX, A]
    lh_ = np.zeros((KMAX, A), np.float32)
    lh_[:k] = best[hidx][:, None] * action_logits[hidx]

    # person-side rhs [NG, KK, NF] block-diagonal (same for all cores)
    rhs = np.zeros((NG, KK, NF), np.float32)
    mug = mu.reshape(NG, GP, A, 4)
    m2g = m2.reshape(NG, GP, A)
    for j in range(GP):
        blk = slice(j * A, (j + 1) * A)
        for cc in range(4):
            rhs[:, cc * GP + j, blk] = mug[:, j, :, cc]
        rhs[:, 4 * GP + j, blk] = 1.0
        rhs[:, 5 * GP + j, blk] = -0.5 * m2g[:, j]
    rhs[:, 6 * GP, :] = 1.0

    lrow = lh_.reshape(NG, NF)
    if _mode() == "fp16hl":
        lrep = np.ascontiguousarray(
            np.broadcast_to(lrow[:, None, :], (NG, P, NF))
        )
        bhi, blo = _hilo(rhs)
        rhs_hh = np.concatenate([bhi, bhi], axis=1)         # [NG, 2KK, NF]
        rhs_lo = blo                                        # [NG, KK, NF]
    else:
        lrep = np.ascontiguousarray(
            np.broadcast_to(lrow[:, None, :], (NG, P, NF))
        )
        rhs_hh = rhs_lo = None

    in_maps = []
    for c in range(NCORES):
        sl = slice(c * OPC, (c + 1) * OPC)
        cxo = np.zeros(NOC, np.float32); cxo[:OPC] = cx_p[sl]
        cyo = np.zeros(NOC, np.float32); cyo[:OPC] = cy_p[sl]
        lwo = np.zeros(NOC, np.float32); lwo[:OPC] = lw_p[sl]
        lho = np.zeros(NOC, np.float32); lho[:OPC] = lh_p[sl]
        lno = np.full(NOC, NEG, np.float32); lno[:OPC] = lnobj_p[sl]

        tx = cxo[None, :] * invw[:, None] - (cxh * invw)[:, None]   # [KMAX, NOC]
        ty = cyo[None, :] * invh[:, None] - (cyh * invh)[:, None]
        tw = lwo[None, :] - lwh[:, None]
        th = lho[None, :] - lhh[:, None]
        tx = np.clip(tx, -TCLAMP, TCLAMP)
        ty = np.clip(ty, -TCLAMP, TCLAMP)
        tw = np.clip(tw, -TCLAMP, TCLAMP)
        th = np.clip(th, -TCLAMP, TCLAMP)
        e2 = tx * tx + ty * ty + tw * tw + th * th

        lhsT = np.zeros((NG, KK, NOC), np.float32)
        g4 = lambda a: a.reshape(NG, GP, NOC)
        lhsT[:, 0:GP] = g4(tx)
        lhsT[:, GP:2 * GP] = g4(ty)
        lhsT[:, 2 * GP:3 * GP] = g4(tw)
        lhsT[:, 3 * GP:4 * GP] = g4(th)
        lhsT[:, 4 * GP:5 * GP] = g4(-0.5 * e2)
        lhsT[:, 5 * GP:6 * GP] = 1.0
        lhsT[:, 6 * GP] = np.maximum(lno / SCALE, LNFLOOR)

        if _mode() == "fp16hl":
            ahi, alo = _hilo(lhsT)
            blob = np.zeros((NG, 2 * KK, NOC + 2 * NF), np.float16)
            blob[:, :KK, :NOC] = ahi
            blob[:, KK:, :NOC] = alo
            blob[:, :, NOC:NOC + NF] = rhs_hh
            blob[:, :KK, NOC + NF:] = rhs_lo
            in_maps.append({"blob": blob, "lrep": lrep})
        else:
            in_maps.append({"lhst": lhsT, "rhs": rhs, "lrep": lrep})
    return in_maps


def _gather(results, k):
    parts = []
    for r in results:
        o = np.asarray(r["out"])
        if _mode() == "fp16hl":
            # [NG, P, NT*NF] -> persons x objects x actions
            o = o.reshape(NG, P, NT, GP, A)
            o = o.transpose(0, 3, 2, 1, 4).reshape(KMAX, NOC, A)
        else:
            o = o.reshape(NG, NT, P, GP, A)
            o = o.transpose(0, 3, 1, 2, 4).reshape(KMAX, NOC, A)
        parts.append(o[:k, :OPC, :])
    return np.concatenate(parts, axis=1)[:, :N, :]


_NC_CACHE = {}


def _build_nc():
    if "nc" in _NC_CACHE:
        return _NC_CACHE["nc"]
    import concourse.bacc as bacc
    import concourse.mybir as mybir
    from concourse.tile import TileContext

    f32 = mybir.dt.float32
    f16 = mybir.dt.float16
    mode = _mode()
    mmdt = mybir.dt.float32r if mode == "f32r" else mybir.dt.float32
    nc = bacc.Bacc()
    if mode == "fp16hl":
        WB = NOC + 2 * NF       # 1280 blob cols
        NW = NT * NF            # 1024 output cols
        blob_d = nc.dram_tensor(
            "blob", [NG, 2 * KK, WB], f16, kind="ExternalInput"
        )
        lrep_d = nc.dram_tensor("lrep", [NG, P, NF], f32, kind="ExternalInput")
        out_d = nc.dram_tensor("out", [NG, P, NW], f32, kind="ExternalOutput")

        with TileContext(nc) as tc:
            with (
                tc.tile_pool(name="wts", bufs=3) as wpool,
                tc.tile_pool(name="work", bufs=3) as work,
                tc.tile_pool(name="mmps", bufs=3, space="PSUM") as pspool,
            ):
                for g in range(NG):
                    blob = wpool.tile([2 * KK, WB], f16, tag="blob")
                    nc.sync.dma_start(blob[:], blob_d[g])
                    lrep = wpool.tile([P, NF], f32, tag="lrep")
                    nc.sync.dma_start(lrep[:], lrep_d[g])

                    ps = pspool.tile([P, NW], f32, tag="mm")
                    for t in range(NT):
                        csl = slice(t * NF, (t + 1) * NF)
                        psl = slice(t * P, (t + 1) * P)
                        nc.tensor.matmul(
                            ps[:, csl], blob[:, psl],
                            blob[:, NOC:NOC + NF],
                            start=True, stop=False,
                        )
                        nc.tensor.matmul(
                            ps[:, csl], blob[0:KK, psl],
                            blob[0:KK, NOC + NF:],
                            start=False, stop=True,
                        )
                        ex = work.tile([P, NF], f32, tag="ex")
                        nc.scalar.activation(
                            ex[:], ps[:, csl], mybir.ActivationFunctionType.Exp,
                            scale=float(SCALE),
                        )
                        ot = work.tile([P, NF], f32, tag="ot")
                        nc.vector.tensor_mul(ot[:], ex[:], lrep[:])
                        nc.sync.dma_start(out_d[g][:, csl], ot[:])
        nc.finalize()
        _NC_CACHE["nc"] = nc
        return nc

    lhst_d = nc.dram_tensor("lhst", [NG, KK, NOC], mmdt, kind="ExternalInput")
    rhs_d = nc.dram_tensor("rhs", [NG, KK, NF], mmdt, kind="ExternalInput")
    lrep_d = nc.dram_tensor("lrep", [NG, P, NF], f32, kind="ExternalInput")
    out_d = nc.dram_tensor("out", [NG, NT, P, NF], f32, kind="ExternalOutput")

    with TileContext(nc) as tc:
        with (
            tc.tile_pool(name="wts", bufs=2) as wpool,
            tc.tile_pool(name="work", bufs=4) as work,
            tc.tile_pool(name="mmps", bufs=4, space="PSUM") as pspool,
        ):
            for g in range(NG):
                lhsT = wpool.tile([KK, NOC], mmdt, tag="lhsT")
                nc.sync.dma_start(lhsT[:], lhst_d[g])
                rhs = wpool.tile([KK, NF], mmdt, tag="rhs")
                nc.sync.dma_start(rhs[:], rhs_d[g])
                lrep = wpool.tile([P, NF], f32, tag="lrep")
                nc.sync.dma_start(lrep[:], lrep_d[g])

                for t in range(NT):
                    ps = pspool.tile([P, NF], f32, tag="mm")
                    sl = slice(t * P, (t + 1) * P)
                    nc.tensor.matmul(
                        ps[:], lhsT[:, sl], rhs[:],
                        start=True, stop=True,
                    )
                    ex = work.tile([P, NF], f32, tag="ex")
                    nc.scalar.activation(
                        ex[:], ps[:], mybir.ActivationFunctionType.Exp,
                        scale=float(SCALE),
                    )
                    ot = work.tile([P, NF], f32, tag="ot")
                    nc.vector.tensor_mul(ot[:], ex[:], lrep[:])
                    nc.sync.dma_start(out_d[g, t], ot[:])
    nc.finalize()
    _NC_CACHE["nc"] = nc
    return nc


def _run_sim(in_maps):
    results = []
    for m in in_maps:
        lrep = m["lrep"]
        if _mode() == "fp16hl":
            out = np.zeros((NG, P, NT * NF), np.float32)
            for g in range(NG):
                b = m["blob"][g].astype(np.float32)         # [2KK, WB]
                a = b[:, :NOC]
                bh = b[:, NOC:NOC + NF]
                bl = b[:KK, NOC + NF:]
                mm = a.T @ bh + a[:KK, :].T @ bl            # [NOC, NF]
                ex = np.exp(np.minimum(SCALE * mm, 0.0).astype(np.float32))
                o = ex * lrep[g][:, :NF][0][None, :]        # [NOC, NF]
                out[g] = o.reshape(NT, P, NF).transpose(1, 0, 2).reshape(
                    P, NT * NF
                )
        else:
            out = np.zeros((NG, NT, P, NF), np.float32)
            for g in range(NG):
                mm = m["lhst"][g].T @ m["rhs"][g]
                ex = np.exp(np.minimum(SCALE * mm, 0.0).astype(np.float32))
                o = ex * lrep[g][0][None, :]
                out[g] = o.reshape(NT, P, NF)
        results.append({"out": out})
    return results


def kernel(action_logits, target_mean, bbox, scores):
    action_logits = np.asarray(action_logits, np.float32)
    target_mean = np.asarray(target_mean, np.float32)
    bbox = np.asarray(bbox, np.float32)
    scores = np.asarray(scores, np.float32)

    person, best, w, h, cx, cy, cx_p, cy_p, lw_p, lh_p, lnobj_p = _obj_arrays(
        bbox, scores
    )
    obj_arr = (cx_p, cy_p, lw_p, lh_p, lnobj_p)
    hidx_all = np.where(person)[0]

    full = np.zeros((N, N, A), np.float32)
    kernel.last_run = None
    for b0 in range(0, len(hidx_all), KMAX):
        hidx = hidx_all[b0:b0 + KMAX]
        in_maps = _host_prep(
            hidx, best, w, h, cx, cy, obj_arr, target_mean, action_logits
        )
        if os.environ.get("KERNEL_SIM") == "1":
            results = _run_sim(in_maps)
        else:
            from concourse.bass_utils import run_bass_kernel_spmd
            nc = _build_nc()
            kw = {}
            if os.environ.get("KERNEL_TRACE") == "1":
                kw = dict(trace=True, trace_cores=list(range(NCORES)))
            r = run_bass_kernel_spmd(
                nc, in_maps, core_ids=list(range(NCORES)), **kw
            )
            results = r.results
            kernel.last_run = r
        full[hidx] = _gather(results, len(hidx))
    return full



# revision 7
# speedup vs baseline: 1.2994x; 1.2994x over previous
import os
import sys

import numpy as np

for _p in ("/opt/trn_rl_repo",):
    if _p not in sys.path and os.path.isdir(_p):
        sys.path.append(_p)

N = 1500
A = 64
STD = 0.3
PERSON_IDX = 2
INV2S2 = 1.0 / (2.0 * STD * STD)
SCALE = 2.0 * INV2S2

NCORES = 8
OPC = 188            # objects per core (8*188 = 1504 >= 1500)
NOBJ = NCORES * OPC

KK = 13              # contraction rows per group (4*2 mu + 2 e2 + 2 lnlrep + 1 lnobj)
KR = 3 * KK          # 39 rows after hi/lo stacking [Ahi;Alo;Ahi] x [Bhi;Bhi;Blo]
KP = 65              # padded contraction rows (>64 keeps PE in plain 128x128 mode)
GCOLS = 128 + OPC    # per-group blob columns (lhsT 128 | rhs 188)
BCOLS = 2 * GCOLS    # per-batch columns (2 groups) = 632
SCOLS = 2 * BCOLS    # per-superstep columns (2 batches) = 1264

TCLAMP = 16.0        # |t| clamp; clamped pairs have exp() == 0 regardless
LNFLOOR = -3000.0    # floor for ln-terms/SCALE rows; exp -> 0, fp16-safe


def _hilo(a):
    hi = a.astype(np.float16)
    lo = (a - hi.astype(np.float32)).astype(np.float16)
    return hi, lo


def _host_prep(hidx, best, w, h, cx, cy, lnobj_p, target_mean, action_logits, ns):
    """Build per-core input blobs.

    Returns (in_maps, sgn) where in_maps[c] = {"blob": [NS, 128, BCOLS] f16}
    and sgn is [NPER, A] signs of humaness*action_logits.
    """
    k = len(hidx)
    nper = ns * 8                      # persons incl. padding
    nb = ns * 2                        # batches of 4 persons

    # per-person params, padded
    mu = np.zeros((nper, A, 4), np.float32)
    mu[:k] = target_mean[hidx]
    m2 = (mu * mu).sum(axis=-1)
    lrep = np.zeros((nper, A), np.float32)
    lrep[:k] = best[hidx][:, None] * action_logits[hidx]
    lnl = np.full((nper, A), LNFLOOR * SCALE, np.float32)
    pos = np.abs(lrep) > 0
    lnl[pos] = np.log(np.abs(lrep[pos]))
    lnrow = np.maximum((lnl - m2 * INV2S2) / SCALE, LNFLOOR)   # [nper, A]
    sgn = np.sign(lrep)

    invw = np.ones(nper, np.float32); invw[:k] = 1.0 / w[hidx]
    invh = np.ones(nper, np.float32); invh[:k] = 1.0 / h[hidx]
    cxh = np.zeros(nper, np.float32); cxh[:k] = cx[hidx]
    cyh = np.zeros(nper, np.float32); cyh[:k] = cy[hidx]
    lwh = np.zeros(nper, np.float32); lwh[:k] = np.log(w[hidx])
    lhh = np.zeros(nper, np.float32); lhh[:k] = np.log(h[hidx])

    # lhsT A [nper_group_pairs...]: built per group of 2 persons
    # A rows [KK, 128] per group; B rows [KK, OPC] per (group, core)
    # padded object arrays
    cx_p = np.zeros(NOBJ, np.float32); cx_p[:N] = cx
    cy_p = np.zeros(NOBJ, np.float32); cy_p[:N] = cy
    lw_p = np.zeros(NOBJ, np.float32); lw_p[:N] = np.log(w)
    lh_p = np.zeros(NOBJ, np.float32); lh_p[:N] = np.log(h)

    # t/e2 for all persons x all (padded) objects
    tx = np.clip(cx_p[None, :] * invw[:, None] - (cxh * invw)[:, None],
                 -TCLAMP, TCLAMP)                                  # [nper, NOBJ]
    ty = np.clip(cy_p[None, :] * invh[:, None] - (cyh * invh)[:, None],
                 -TCLAMP, TCLAMP)
    tw = np.clip(lw_p[None, :] - lwh[:, None], -TCLAMP, TCLAMP)
    th = np.clip(lh_p[None, :] - lhh[:, None], -TCLAMP, TCLAMP)
    e2 = tx * tx + ty * ty + tw * tw + th * th

    # A [ngroups, KK, 128], partition q = j*64 + a
    ng = nper // 2
    Af = np.zeros((ng, KK, 2, A), np.float32)
    mug = mu.reshape(ng, 2, A, 4)
    lng = lnrow.reshape(ng, 2, A)
    for j in range(2):
        for c in range(4):
            Af[:, c * 2 + j, j, :] = mug[:, j, :, c]
        Af[:, 8 + j, j, :] = 1.0
        Af[:, 10 + j, j, :] = lng[:, j, :]
    Af[:, 12, :, :] = 1.0
    Af = Af.reshape(ng, KK, 128)
    Ahi, Alo = _hilo(Af)
    A39 = np.concatenate([Ahi, Alo, Ahi], axis=1)      # [ng, KR, 128]

    # B [ngroups, KK, NOBJ]
    Bf = np.zeros((ng, KK, NOBJ), np.float32)
    g2 = lambda x: x.reshape(ng, 2, NOBJ)
    txg, tyg, twg, thg, e2g = g2(tx), g2(ty), g2(tw), g2(th), g2(e2)
    for j in range(2):
        for c, tc in enumerate((txg, tyg, twg, thg)):
            Bf[:, c * 2 + j, :] = tc[:, j, :]
        Bf[:, 8 + j, :] = -0.5 * e2g[:, j, :]
        Bf[:, 10 + j, :] = 1.0
    lnobj_row = np.maximum(lnobj_p / SCALE, LNFLOOR)
    Bf[:, 12, :] = lnobj_row[None, :]
    Bhi, Blo = _hilo(Bf)
    B39 = np.concatenate([Bhi, Bhi, Blo], axis=1)      # [ng, KR, NOBJ]

    in_maps = []
    for c in range(NCORES):
        osl = slice(c * OPC, (c + 1) * OPC)
        blob = np.zeros((ns, KP, SCOLS), np.float16)
        for b in range(nb):
            s, dd = divmod(b, 2)
            for g01 in range(2):
                g = b * 2 + g01
                col0 = dd * BCOLS + g01 * GCOLS
                blob[s, :KR, col0:col0 + 128] = A39[g]
                blob[s, :KR, col0 + 128:col0 + 128 + OPC] = B39[g][:, osl]
        in_maps.append({"blob": blob})
    return in_maps, sgn


_NC_CACHE = {}


def _build_nc(ns):
    if ns in _NC_CACHE:
        return _NC_CACHE[ns]
    import concourse.bacc as bacc
    import concourse.mybir as mybir
    from concourse.tile import TileContext

    f32 = mybir.dt.float32
    f16 = mybir.dt.float16
    nc = bacc.Bacc()
    blob_d = nc.dram_tensor("blob", [ns, KP, SCOLS], f16, kind="ExternalInput")
    out_d = nc.dram_tensor("out", [ns, 128, 4, OPC], f16, kind="ExternalOutput")

    with TileContext(nc) as tc:
        with (
            tc.tile_pool(name="io", bufs=3) as iop,
            tc.tile_pool(name="mmps", bufs=2, space="PSUM") as psp,
        ):
            tins = []
            for s in range(ns):
                tin = iop.tile([KP, SCOLS], f16, tag=f"tin{s}", bufs=1)
                nc.sync.dma_start(tin[:], blob_d[s])
                tins.append(tin)
            for s in range(ns):
                tin = tins[s]
                ps = psp.tile([128, 4, 512], f32, tag="mm")
                for dd in range(2):
                    for g01 in range(2):
                        col0 = dd * BCOLS + g01 * GCOLS
                        nc.tensor.matmul(
                            ps[:, 2 * dd + g01, 0:OPC],
                            tin[0:KP, col0:col0 + 128],
                            tin[0:KP, col0 + 128:col0 + 128 + OPC],
                            start=True, stop=True,
                        )
                ot = iop.tile([128, 4, OPC], f16, tag="ot")
                nc.scalar.activation(
                    ot[:], ps[:, :, 0:OPC],
                    mybir.ActivationFunctionType.Exp, scale=float(SCALE),
                )
                nc.sync.dma_start(out_d[s], ot[:])
    nc.finalize()
    _NC_CACHE[ns] = nc
    return nc


def _run_sim(in_maps, ns):
    """Numpy emulation of the device program (incl. fp16 rounding)."""
    results = []
    for m in in_maps:
        blob = m["blob"].astype(np.float32)
        out = np.zeros((ns, 128, 4, OPC), np.float32)
        for s in range(ns):
            for dd in range(2):
                for g01 in range(2):
                    col0 = dd * BCOLS + g01 * GCOLS
                    a = blob[s, :KR, col0:col0 + 128]
                    b = blob[s, :KR, col0 + 128:col0 + 128 + OPC]
                    mm = a.T @ b
                    out[s, :, 2 * dd + g01, :] = np.exp(
                        np.minimum(SCALE * mm, 80.0))
        results.append({"out": out.astype(np.float16)})
    return results


def _gather(results, ns, k, sgn):
    nper = ns * 8
    parts = []
    for r in results:
        o = np.asarray(r["out"]).astype(np.float32)   # [ns, 128, 4, OPC]
        # partition q = j*64 + a ; bank = 2*dd + g01 ; person = b*4+g01*2+j
        o = o.reshape(ns, 2, A, 2, 2, OPC)            # s, j, a, dd, g01, o
        o = o.transpose(0, 3, 4, 1, 5, 2)             # s, dd, g01, j, o, a
        parts.append(o.reshape(nper, OPC, A))
    full_obj = np.concatenate(parts, axis=1)          # [nper, NOBJ, A]
    return full_obj[:k, :N, :] * sgn[:k, None, :]


def kernel(action_logits, target_mean, bbox, scores):
    action_logits = np.asarray(action_logits, np.float32)
    target_mean = np.asarray(target_mean, np.float32)
    bbox = np.asarray(bbox, np.float32)
    scores = np.asarray(scores, np.float32)

    best = scores.max(axis=1)
    idx = scores.argmax(axis=1)
    person = idx == PERSON_IDX
    obj = np.where(person, 0.0, best).astype(np.float32)

    w = bbox[:, 2] - bbox[:, 0]
    h = bbox[:, 3] - bbox[:, 1]
    cx = bbox[:, 0] + 0.5 * w
    cy = bbox[:, 1] + 0.5 * h

    lnobj_p = np.full(NOBJ, LNFLOOR * SCALE, np.float32)
    pos = obj > 0
    lnobj_p[:N][pos] = np.log(obj[pos])

    hidx = np.where(person)[0]
    k = len(hidx)
    full = np.zeros((N, N, A), np.float32)
    kernel.last_run = None
    if k == 0:
        return full

    ns = max(1, (k + 7) // 8)          # supersteps of 8 persons
    in_maps, sgn = _host_prep(
        hidx, best, w, h, cx, cy, lnobj_p, target_mean, action_logits, ns
    )
    if os.environ.get("KERNEL_SIM") == "1":
        results = _run_sim(in_maps, ns)
    else:
        from concourse.bass_utils import run_bass_kernel_spmd
        nc = _build_nc(ns)
        kw = {}
        if os.environ.get("KERNEL_TRACE") == "1":
            kw = dict(trace=True, trace_cores=list(range(NCORES)))
        r = run_bass_kernel_spmd(
            nc, in_maps, core_ids=list(range(NCORES)), **kw
        )
        results = r.results
        kernel.last_run = r
    full[hidx] = _gather(results, ns, k, sgn)
    return full


# revision 15
# speedup vs baseline: 1.3612x; 1.0475x over previous
import os
import sys

import numpy as np

for _p in ("/opt/trn_rl_repo",):
    if _p not in sys.path and os.path.isdir(_p):
        sys.path.append(_p)

N = 1500
A = 64
STD = 0.3
PERSON_IDX = 2
INV2S2 = 1.0 / (2.0 * STD * STD)
SCALE = 2.0 * INV2S2

NCORES = 8
OPC = 188            # objects per core (8*188 = 1504 >= 1500)
NOBJ = NCORES * OPC

KK = 13              # contraction rows per group (4*2 mu + 2 e2 + 2 lnlrep + 1 lnobj)
KR = 3 * KK          # 39 rows after hi/lo stacking [Ahi;Alo;Ahi] x [Bhi;Bhi;Blo]
KP = 65              # padded contraction rows (>64 keeps PE in plain 128x128 mode)
PAD = KP - KR        # 26 zero rows, at partitions 0:PAD (memset must start at 0)
GCOLS = 128 + OPC    # per-group blob columns (lhsT 128 | rhs 188)
BCOLS = 2 * GCOLS    # per-batch columns (2 groups) = 632
SCOLS = 2 * BCOLS    # per-superstep columns (2 batches) = 1264

TCLAMP = 16.0        # |t| clamp; clamped pairs have exp() == 0 regardless
LNFLOOR = -3000.0    # floor for ln-terms/SCALE rows; exp -> 0, fp16-safe


def _hilo(a):
    hi = a.astype(np.float16)
    lo = (a - hi.astype(np.float32)).astype(np.float16)
    return hi, lo


def _host_prep(hidx, best, w, h, cx, cy, lnobj_p, target_mean, action_logits, ns):
    """Build per-core input blobs.

    Returns (in_maps, sgn) where in_maps[c] = {"blob": [NS, 128, BCOLS] f16}
    and sgn is [NPER, A] signs of humaness*action_logits.
    """
    k = len(hidx)
    nper = ns * 8                      # persons incl. padding
    nb = ns * 2                        # batches of 4 persons

    # per-person params, padded
    mu = np.zeros((nper, A, 4), np.float32)
    mu[:k] = target_mean[hidx]
    m2 = (mu * mu).sum(axis=-1)
    lrep = np.zeros((nper, A), np.float32)
    lrep[:k] = best[hidx][:, None] * action_logits[hidx]
    lnl = np.full((nper, A), LNFLOOR * SCALE, np.float32)
    pos = np.abs(lrep) > 0
    lnl[pos] = np.log(np.abs(lrep[pos]))
    lnrow = np.maximum((lnl - m2 * INV2S2) / SCALE, LNFLOOR)   # [nper, A]
    sgn = np.sign(lrep)

    invw = np.ones(nper, np.float32); invw[:k] = 1.0 / w[hidx]
    invh = np.ones(nper, np.float32); invh[:k] = 1.0 / h[hidx]
    cxh = np.zeros(nper, np.float32); cxh[:k] = cx[hidx]
    cyh = np.zeros(nper, np.float32); cyh[:k] = cy[hidx]
    lwh = np.zeros(nper, np.float32); lwh[:k] = np.log(w[hidx])
    lhh = np.zeros(nper, np.float32); lhh[:k] = np.log(h[hidx])

    # lhsT A [nper_group_pairs...]: built per group of 2 persons
    # A rows [KK, 128] per group; B rows [KK, OPC] per (group, core)
    # padded object arrays
    cx_p = np.zeros(NOBJ, np.float32); cx_p[:N] = cx
    cy_p = np.zeros(NOBJ, np.float32); cy_p[:N] = cy
    lw_p = np.zeros(NOBJ, np.float32); lw_p[:N] = np.log(w)
    lh_p = np.zeros(NOBJ, np.float32); lh_p[:N] = np.log(h)

    # t/e2 for all persons x all (padded) objects
    tx = np.clip(cx_p[None, :] * invw[:, None] - (cxh * invw)[:, None],
                 -TCLAMP, TCLAMP)                                  # [nper, NOBJ]
    ty = np.clip(cy_p[None, :] * invh[:, None] - (cyh * invh)[:, None],
                 -TCLAMP, TCLAMP)
    tw = np.clip(lw_p[None, :] - lwh[:, None], -TCLAMP, TCLAMP)
    th = np.clip(lh_p[None, :] - lhh[:, None], -TCLAMP, TCLAMP)
    e2 = tx * tx + ty * ty + tw * tw + th * th

    # A [ngroups, KK, 128], partition q = j*64 + a
    ng = nper // 2
    Af = np.zeros((ng, KK, 2, A), np.float32)
    mug = mu.reshape(ng, 2, A, 4)
    lng = lnrow.reshape(ng, 2, A)
    for j in range(2):
        for c in range(4):
            Af[:, c * 2 + j, j, :] = mug[:, j, :, c]
        Af[:, 8 + j, j, :] = 1.0
        Af[:, 10 + j, j, :] = lng[:, j, :]
    Af[:, 12, :, :] = 1.0
    Af = Af.reshape(ng, KK, 128)
    Ahi, Alo = _hilo(Af)
    A39 = np.concatenate([Ahi, Alo, Ahi], axis=1)      # [ng, KR, 128]

    # B [ngroups, KK, NOBJ]
    Bf = np.zeros((ng, KK, NOBJ), np.float32)
    g2 = lambda x: x.reshape(ng, 2, NOBJ)
    txg, tyg, twg, thg, e2g = g2(tx), g2(ty), g2(tw), g2(th), g2(e2)
    for j in range(2):
        for c, tc in enumerate((txg, tyg, twg, thg)):
            Bf[:, c * 2 + j, :] = tc[:, j, :]
        Bf[:, 8 + j, :] = -0.5 * e2g[:, j, :]
        Bf[:, 10 + j, :] = 1.0
    lnobj_row = np.maximum(lnobj_p / SCALE, LNFLOOR)
    Bf[:, 12, :] = lnobj_row[None, :]
    Bhi, Blo = _hilo(Bf)
    B39 = np.concatenate([Bhi, Bhi, Blo], axis=1)      # [ng, KR, NOBJ]

    in_maps = []
    for c in range(NCORES):
        osl = slice(c * OPC, (c + 1) * OPC)
        blob = np.zeros((ns, KR, SCOLS), np.float16)
        for b in range(nb):
            s, dd = divmod(b, 2)
            for g01 in range(2):
                g = b * 2 + g01
                col0 = dd * BCOLS + g01 * GCOLS
                blob[s, :, col0:col0 + 128] = A39[g]
                blob[s, :, col0 + 128:col0 + 128 + OPC] = B39[g][:, osl]
        in_maps.append({"blob": blob})
    return in_maps, sgn


_NC_CACHE = {}


def _build_nc(ns):
    """Raw-bass program (no TileContext): minimal semaphores, minimal
    preamble/teardown.  Per superstep s (8 persons): one input DMA, four
    matmuls (one per person pair) into separate PSUM banks, one Exp
    activation reading all four banks, one output DMA."""
    if ns in _NC_CACHE:
        return _NC_CACHE[ns]
    import concourse.bacc as bacc
    import concourse.mybir as mybir

    f32 = mybir.dt.float32
    f16 = mybir.dt.float16
    nc = bacc.Bacc()
    blob_d = nc.dram_tensor("blob", [ns, KR, SCOLS], f16, kind="ExternalInput")
    out_d = nc.dram_tensor("out", [ns, 128, 4, OPC], f16, kind="ExternalOutput")

    tins = [nc.alloc_sbuf_tensor(f"tin{s}", [KP, SCOLS], f16) for s in range(ns)]
    ots = [nc.alloc_sbuf_tensor(f"ot{s}", [128, 4, OPC], f16) for s in range(ns)]
    bias = nc.alloc_sbuf_tensor("bias", [128, 1], f32)
    pss = [nc.alloc_psum_tensor(f"ps{i}", [128, 4, 512], f32) for i in range(2)]

    s_ins = [nc.alloc_semaphore(f"s_in{s}") for s in range(ns)]
    s_ms = nc.alloc_semaphore("s_ms")
    s_mm = nc.alloc_semaphore("s_mm")
    s_act = nc.alloc_semaphore("s_act")
    s_out = nc.alloc_semaphore("s_out")

    with nc.Block() as block:

        @block.vector
        def _(v):
            v.memset(bias[:, :], 0.0)
            for s in range(ns):
                v.memset(tins[s][0:PAD, :], 0.0).then_inc(s_ms)

        @block.sync
        def _(sp):
            for s in range(ns):
                sp.dma_start(tins[s][PAD:KP, :], blob_d[s]).then_inc(s_ins[s], 16)
            for s in range(ns):
                sp.wait_ge(s_act, s + 1)
                sp.dma_start(out_d[s], ots[s][:]).then_inc(s_out, 16)
            sp.wait_ge(s_out, 16 * ns)

        @block.tensor
        def _(te):
            for s in range(ns):
                te.wait_ge(s_ms, s + 1)
                te.wait_ge(s_ins[s], 16)
                if s >= 2:
                    te.wait_ge(s_act, s - 1)   # psum bank reuse (s-2's act)
                ps = pss[s % 2]
                for dd in range(2):
                    for g01 in range(2):
                        col0 = dd * BCOLS + g01 * GCOLS
                        te.matmul(
                            ps[:, 2 * dd + g01, 0:OPC],
                            tins[s][0:KP, col0:col0 + 128],
                            tins[s][0:KP, col0 + 128:col0 + 128 + OPC],
                            start=True, stop=True,
                        ).then_inc(s_mm)

        @block.scalar
        def _(sc):
            sc.wait_ge(s_ms, 1)
            for s in range(ns):
                sc.wait_ge(s_mm, 4 * (s + 1))
                sc.activation(
                    ots[s][:], pss[s % 2][:, :, 0:OPC],
                    mybir.ActivationFunctionType.Exp,
                    bias=bias[:, 0:1], scale=float(SCALE),
                ).then_inc(s_act)

    nc.finalize()
    _NC_CACHE[ns] = nc
    return nc


def _run_sim(in_maps, ns):
    """Numpy emulation of the device program (incl. fp16 rounding)."""
    results = []
    for m in in_maps:
        blob = m["blob"].astype(np.float32)
        out = np.zeros((ns, 128, 4, OPC), np.float32)
        for s in range(ns):
            for dd in range(2):
                for g01 in range(2):
                    col0 = dd * BCOLS + g01 * GCOLS
                    a = blob[s, :, col0:col0 + 128]
                    b = blob[s, :, col0 + 128:col0 + 128 + OPC]
                    mm = a.T @ b
                    out[s, :, 2 * dd + g01, :] = np.exp(
                        np.minimum(SCALE * mm, 80.0))
        results.append({"out": out.astype(np.float16)})
    return results


def _gather(results, ns, k, sgn):
    nper = ns * 8
    parts = []
    for r in results:
        o = np.asarray(r["out"]).astype(np.float32)   # [ns, 128, 4, OPC]
        # partition q = j*64 + a ; bank = 2*dd + g01 ; person = b*4+g01*2+j
        o = o.reshape(ns, 2, A, 2, 2, OPC)            # s, j, a, dd, g01, o
        o = o.transpose(0, 3, 4, 1, 5, 2)             # s, dd, g01, j, o, a
        parts.append(o.reshape(nper, OPC, A))
    full_obj = np.concatenate(parts, axis=1)          # [nper, NOBJ, A]
    return full_obj[:k, :N, :] * sgn[:k, None, :]


def kernel(action_logits, target_mean, bbox, scores):
    action_logits = np.asarray(action_logits, np.float32)
    target_mean = np.asarray(target_mean, np.float32)
    bbox = np.asarray(bbox, np.float32)
    scores = np.asarray(scores, np.float32)

    best = scores.max(axis=1)
    idx = scores.argmax(axis=1)
    person = idx == PERSON_IDX
    obj = np.where(person, 0.0, best).astype(np.float32)

    w = bbox[:, 2] - bbox[:, 0]
    h = bbox[:, 3] - bbox[:, 1]
    cx = bbox[:, 0] + 0.5 * w
    cy = bbox[:, 1] + 0.5 * h

    lnobj_p = np.full(NOBJ, LNFLOOR * SCALE, np.float32)
    pos = obj > 0
    lnobj_p[:N][pos] = np.log(obj[pos])

    hidx = np.where(person)[0]
    k = len(hidx)
    full = np.zeros((N, N, A), np.float32)
    kernel.last_run = None
    if k == 0:
        return full

    ns = max(1, (k + 7) // 8)          # supersteps of 8 persons
    in_maps, sgn = _host_prep(
        hidx, best, w, h, cx, cy, lnobj_p, target_mean, action_logits, ns
    )
    if os.environ.get("KERNEL_SIM") == "1":
        results = _run_sim(in_maps, ns)
    else:
        from concourse.bass_utils import run_bass_kernel_spmd
        nc = _build_nc(ns)
        kw = {}
        if os.environ.get("KERNEL_TRACE") == "1":
            kw = dict(trace=True, trace_cores=list(range(NCORES)))
        r = run_bass_kernel_spmd(
            nc, in_maps, core_ids=list(range(NCORES)), **kw
        )
        results = r.results
        kernel.last_run = r
    full[hidx] = _gather(results, ns, k, sgn)
    return full


# revision 17
# speedup vs baseline: 1.4499x; 1.0651x over previous
import os
import sys

import numpy as np

for _p in ("/opt/trn_rl_repo",):
    if _p not in sys.path and os.path.isdir(_p):
        sys.path.append(_p)

N = 1500
A = 64
STD = 0.3
PERSON_IDX = 2
INV2S2 = 1.0 / (2.0 * STD * STD)
SCALE = 2.0 * INV2S2

NCORES = 8
OPC = 188            # objects per core (8*188 = 1504 >= 1500)
NOBJ = NCORES * OPC

KK = 13              # contraction rows per group (4*2 mu + 2 e2 + 2 lnlrep + 1 lnobj)
KR = 3 * KK          # 39 rows after hi/lo stacking [Ahi;Alo;Ahi] x [Bhi;Bhi;Blo]
KP = 65              # padded contraction rows (>64 keeps PE in plain 128x128 mode)
PAD = KP - KR        # 26 zero rows, at partitions 0:PAD (memset must start at 0)
GCOLS = 128 + OPC    # per-group blob columns (lhsT 128 | rhs 188)
BCOLS = 2 * GCOLS    # per-batch columns (2 groups) = 632
SCOLS = 2 * BCOLS    # per-superstep columns (2 batches) = 1264

TCLAMP = 16.0        # |t| clamp; clamped pairs have exp() == 0 regardless
LNFLOOR = -3000.0    # floor for ln-terms/SCALE rows; exp -> 0, fp16-safe


def _hilo(a):
    hi = a.astype(np.float16)
    lo = (a - hi.astype(np.float32)).astype(np.float16)
    return hi, lo


def _host_prep(hidx, best, w, h, cx, cy, lnobj_p, target_mean, action_logits, ns):
    """Build per-core input blobs.

    Returns (in_maps, sgn) where in_maps[c] = {"blob": [NS, 128, BCOLS] f16}
    and sgn is [NPER, A] signs of humaness*action_logits.
    """
    k = len(hidx)
    nper = ns * 8                      # persons incl. padding
    nb = ns * 2                        # batches of 4 persons

    # per-person params, padded
    mu = np.zeros((nper, A, 4), np.float32)
    mu[:k] = target_mean[hidx]
    m2 = (mu * mu).sum(axis=-1)
    lrep = np.zeros((nper, A), np.float32)
    lrep[:k] = best[hidx][:, None] * action_logits[hidx]
    lnl = np.full((nper, A), LNFLOOR * SCALE, np.float32)
    pos = np.abs(lrep) > 0
    lnl[pos] = np.log(np.abs(lrep[pos]))
    lnrow = np.maximum((lnl - m2 * INV2S2) / SCALE, LNFLOOR)   # [nper, A]
    sgn = np.sign(lrep)

    invw = np.ones(nper, np.float32); invw[:k] = 1.0 / w[hidx]
    invh = np.ones(nper, np.float32); invh[:k] = 1.0 / h[hidx]
    cxh = np.zeros(nper, np.float32); cxh[:k] = cx[hidx]
    cyh = np.zeros(nper, np.float32); cyh[:k] = cy[hidx]
    lwh = np.zeros(nper, np.float32); lwh[:k] = np.log(w[hidx])
    lhh = np.zeros(nper, np.float32); lhh[:k] = np.log(h[hidx])

    # lhsT A [nper_group_pairs...]: built per group of 2 persons
    # A rows [KK, 128] per group; B rows [KK, OPC] per (group, core)
    # padded object arrays
    cx_p = np.zeros(NOBJ, np.float32); cx_p[:N] = cx
    cy_p = np.zeros(NOBJ, np.float32); cy_p[:N] = cy
    lw_p = np.zeros(NOBJ, np.float32); lw_p[:N] = np.log(w)
    lh_p = np.zeros(NOBJ, np.float32); lh_p[:N] = np.log(h)

    # t/e2 for all persons x all (padded) objects
    tx = np.clip(cx_p[None, :] * invw[:, None] - (cxh * invw)[:, None],
                 -TCLAMP, TCLAMP)                                  # [nper, NOBJ]
    ty = np.clip(cy_p[None, :] * invh[:, None] - (cyh * invh)[:, None],
                 -TCLAMP, TCLAMP)
    tw = np.clip(lw_p[None, :] - lwh[:, None], -TCLAMP, TCLAMP)
    th = np.clip(lh_p[None, :] - lhh[:, None], -TCLAMP, TCLAMP)
    e2 = tx * tx + ty * ty + tw * tw + th * th

    # A [ngroups, KK, 128], partition q = j*64 + a
    ng = nper // 2
    Af = np.zeros((ng, KK, 2, A), np.float32)
    mug = mu.reshape(ng, 2, A, 4)
    lng = lnrow.reshape(ng, 2, A)
    for j in range(2):
        for c in range(4):
            Af[:, c * 2 + j, j, :] = mug[:, j, :, c]
        Af[:, 8 + j, j, :] = 1.0
        Af[:, 10 + j, j, :] = lng[:, j, :]
    Af[:, 12, :, :] = 1.0
    Af = Af.reshape(ng, KK, 128)
    Ahi, Alo = _hilo(Af)
    A39 = np.concatenate([Ahi, Alo, Ahi], axis=1)      # [ng, KR, 128]

    # B [ngroups, KK, NOBJ]
    Bf = np.zeros((ng, KK, NOBJ), np.float32)
    g2 = lambda x: x.reshape(ng, 2, NOBJ)
    txg, tyg, twg, thg, e2g = g2(tx), g2(ty), g2(tw), g2(th), g2(e2)
    for j in range(2):
        for c, tc in enumerate((txg, tyg, twg, thg)):
            Bf[:, c * 2 + j, :] = tc[:, j, :]
        Bf[:, 8 + j, :] = -0.5 * e2g[:, j, :]
        Bf[:, 10 + j, :] = 1.0
    lnobj_row = np.maximum(lnobj_p / SCALE, LNFLOOR)
    Bf[:, 12, :] = lnobj_row[None, :]
    Bhi, Blo = _hilo(Bf)
    B39 = np.concatenate([Bhi, Bhi, Blo], axis=1)      # [ng, KR, NOBJ]

    in_maps = []
    for c in range(NCORES):
        osl = slice(c * OPC, (c + 1) * OPC)
        blob = np.zeros((ns, KR, SCOLS), np.float16)
        for b in range(nb):
            s, dd = divmod(b, 2)
            for g01 in range(2):
                g = b * 2 + g01
                col0 = dd * BCOLS + g01 * GCOLS
                blob[s, :, col0:col0 + 128] = A39[g]
                blob[s, :, col0 + 128:col0 + 128 + OPC] = B39[g][:, osl]
        in_maps.append({"blob": blob})
    return in_maps, sgn


_NC_CACHE = {}


def _build_nc(ns):
    """Raw-bass program (no TileContext): minimal semaphores, minimal
    preamble/teardown.  Per superstep s (8 persons): one input DMA, four
    matmuls (one per person pair) into separate PSUM banks, one Exp
    activation reading all four banks, one output DMA."""
    if ns in _NC_CACHE:
        return _NC_CACHE[ns]
    import concourse.bacc as bacc
    import concourse.mybir as mybir

    f32 = mybir.dt.float32
    f16 = mybir.dt.float16
    nc = bacc.Bacc()
    blob_d = nc.dram_tensor("blob", [ns, KR, SCOLS], f16, kind="ExternalInput")
    out_d = nc.dram_tensor("out", [ns, 128, 4, OPC], f16, kind="ExternalOutput")

    tins = [nc.alloc_sbuf_tensor(f"tin{s}", [KP, SCOLS], f16) for s in range(ns)]
    ots = [nc.alloc_sbuf_tensor(f"ot{s}", [128, 4, OPC], f16) for s in range(ns)]
    bias = nc.alloc_sbuf_tensor("bias", [128, 1], f32)
    scr = nc.alloc_sbuf_tensor("scr", [128, 1], f16)
    zbuf = nc.alloc_sbuf_tensor("zbuf", [128, 640], f16)
    pss = [nc.alloc_psum_tensor(f"ps{i}", [128, 4, 512], f32) for i in range(2)]

    s_ins = [nc.alloc_semaphore(f"s_in{s}") for s in range(ns)]
    s_ms = nc.alloc_semaphore("s_ms")
    s_mm = nc.alloc_semaphore("s_mm")
    s_act = nc.alloc_semaphore("s_act")
    s_out = nc.alloc_semaphore("s_out")

    with nc.Block() as block:

        @block.vector
        def _(v):
            v.memset(bias[:, :], 0.0).then_inc(s_ms)       # s_ms: 1
            v.memset(zbuf[:, :], 0.0).then_inc(s_ms)       # s_ms: 2
            for s in range(ns):
                v.memset(tins[s][0:PAD, :], 0.0).then_inc(s_ms)  # 3 + s

        @block.scalar
        def _(sc):
            # input DMAs on the ACT HWDGE ring (own ring; output ring stays
            # free for stores)
            for s in range(ns):
                sc.dma_start(tins[s][PAD:KP, :], blob_d[s]).then_inc(s_ins[s], 16)
            # dummy activation: pulls the EXP table load off the critical path
            sc.wait_ge(s_ms, 1)
            sc.activation(
                scr[:, :], bias[:, 0:1],
                mybir.ActivationFunctionType.Exp,
                bias=bias[:, 0:1], scale=1.0,
            )
            for s in range(ns):
                sc.wait_ge(s_mm, 4 * (s + 1))
                sc.activation(
                    ots[s][:], pss[s % 2][:, :, 0:OPC],
                    mybir.ActivationFunctionType.Exp,
                    bias=bias[:, 0:1], scale=float(SCALE),
                ).then_inc(s_act)
            # second half of the last output store, on this ring
            sc.wait_ge(s_act, ns)
            sc.dma_start(
                out_d[ns - 1][:, 2:4], ots[ns - 1][:, 2:4]
            ).then_inc(s_out, 16)

        @block.tensor
        def _(te):
            # warm-up matmuls on zeroed SBUF (ramp the PE clock)
            te.wait_ge(s_ms, 2)
            for _ in range(3):
                te.matmul(
                    pss[1][:, 3, 0:512], zbuf[0:KP, 0:128], zbuf[0:KP, 128:640],
                    start=True, stop=True,
                )
            for s in range(ns):
                te.wait_ge(s_ms, s + 3)
                te.wait_ge(s_ins[s], 16)
                if s >= 2:
                    te.wait_ge(s_act, s - 1)   # psum bank reuse (s-2's act)
                ps = pss[s % 2]
                for dd in range(2):
                    for g01 in range(2):
                        col0 = dd * BCOLS + g01 * GCOLS
                        te.matmul(
                            ps[:, 2 * dd + g01, 0:OPC],
                            tins[s][0:KP, col0:col0 + 128],
                            tins[s][0:KP, col0 + 128:col0 + 128 + OPC],
                            start=True, stop=True,
                        ).then_inc(s_mm)

        @block.sync
        def _(sp):
            for s in range(ns):
                sp.wait_ge(s_act, s + 1)
                if s == ns - 1:
                    sp.dma_start(out_d[s][:, 0:2], ots[s][:, 0:2]).then_inc(
                        s_out, 16
                    )
                else:
                    sp.dma_start(out_d[s], ots[s][:]).then_inc(s_out, 16)
            sp.wait_ge(s_out, 16 * (ns + 1))

    nc.finalize()
    _NC_CACHE[ns] = nc
    return nc


def _run_sim(in_maps, ns):
    """Numpy emulation of the device program (incl. fp16 rounding)."""
    results = []
    for m in in_maps:
        blob = m["blob"].astype(np.float32)
        out = np.zeros((ns, 128, 4, OPC), np.float32)
        for s in range(ns):
            for dd in range(2):
                for g01 in range(2):
                    col0 = dd * BCOLS + g01 * GCOLS
                    a = blob[s, :, col0:col0 + 128]
                    b = blob[s, :, col0 + 128:col0 + 128 + OPC]
                    mm = a.T @ b
                    out[s, :, 2 * dd + g01, :] = np.exp(
                        np.minimum(SCALE * mm, 80.0))
        results.append({"out": out.astype(np.float16)})
    return results


def _gather(results, ns, k, sgn):
    nper = ns * 8
    parts = []
    for r in results:
        o = np.asarray(r["out"]).astype(np.float32)   # [ns, 128, 4, OPC]
        # partition q = j*64 + a ; bank = 2*dd + g01 ; person = b*4+g01*2+j
        o = o.reshape(ns, 2, A, 2, 2, OPC)            # s, j, a, dd, g01, o
        o = o.transpose(0, 3, 4, 1, 5, 2)             # s, dd, g01, j, o, a
        parts.append(o.reshape(nper, OPC, A))
    full_obj = np.concatenate(parts, axis=1)          # [nper, NOBJ, A]
    return full_obj[:k, :N, :] * sgn[:k, None, :]


def kernel(action_logits, target_mean, bbox, scores):
    action_logits = np.asarray(action_logits, np.float32)
    target_mean = np.asarray(target_mean, np.float32)
    bbox = np.asarray(bbox, np.float32)
    scores = np.asarray(scores, np.float32)

    best = scores.max(axis=1)
    idx = scores.argmax(axis=1)
    person = idx == PERSON_IDX
    obj = np.where(person, 0.0, best).astype(np.float32)

    w = bbox[:, 2] - bbox[:, 0]
    h = bbox[:, 3] - bbox[:, 1]
    cx = bbox[:, 0] + 0.5 * w
    cy = bbox[:, 1] + 0.5 * h

    lnobj_p = np.full(NOBJ, LNFLOOR * SCALE, np.float32)
    pos = obj > 0
    lnobj_p[:N][pos] = np.log(obj[pos])

    hidx = np.where(person)[0]
    k = len(hidx)
    full = np.zeros((N, N, A), np.float32)
    kernel.last_run = None
    if k == 0:
        return full

    ns = max(1, (k + 7) // 8)          # supersteps of 8 persons
    in_maps, sgn = _host_prep(
        hidx, best, w, h, cx, cy, lnobj_p, target_mean, action_logits, ns
    )
    if os.environ.get("KERNEL_SIM") == "1":
        results = _run_sim(in_maps, ns)
    else:
        from concourse.bass_utils import run_bass_kernel_spmd
        nc = _build_nc(ns)
        kw = {}
        if os.environ.get("KERNEL_TRACE") == "1":
            kw = dict(trace=True, trace_cores=list(range(NCORES)))
        r = run_bass_kernel_spmd(
            nc, in_maps, core_ids=list(range(NCORES)), **kw
        )
        results = r.results
        kernel.last_run = r
    full[hidx] = _gather(results, ns, k, sgn)
    return full


# revision 19
# speedup vs baseline: 1.4662x; 1.0113x over previous
import os
import sys

import numpy as np

for _p in ("/opt/trn_rl_repo",):
    if _p not in sys.path and os.path.isdir(_p):
        sys.path.append(_p)

N = 1500
A = 64
STD = 0.3
PERSON_IDX = 2
INV2S2 = 1.0 / (2.0 * STD * STD)
SCALE = 2.0 * INV2S2

NCORES = 8
OPC = 188            # objects per core (8*188 = 1504 >= 1500)
NOBJ = NCORES * OPC

KK = 13              # contraction rows per group (4*2 mu + 2 e2 + 2 lnlrep + 1 lnobj)
KR = 3 * KK          # 39 rows after hi/lo stacking [Ahi;Alo;Ahi] x [Bhi;Bhi;Blo]
KP = 65              # padded contraction rows (>64 keeps PE in plain 128x128 mode)
PAD = KP - KR        # 26 zero rows, at partitions 0:PAD (memset must start at 0)
GCOLS = 128 + OPC    # per-group blob columns (lhsT 128 | rhs 188)
BCOLS = 2 * GCOLS    # per-batch columns (2 groups) = 632
SCOLS = 2 * BCOLS    # per-superstep columns (2 batches) = 1264

TCLAMP = 16.0        # |t| clamp; clamped pairs have exp() == 0 regardless
LNFLOOR = -3000.0    # floor for ln-terms/SCALE rows; exp -> 0, fp16-safe


def _hilo(a):
    hi = a.astype(np.float16)
    lo = (a - hi.astype(np.float32)).astype(np.float16)
    return hi, lo


def _host_prep(hidx, best, w, h, cx, cy, lnobj_p, target_mean, action_logits, ns):
    """Build per-core input blobs.

    Returns (in_maps, sgn) where in_maps[c] = {"blob": [NS, 128, BCOLS] f16}
    and sgn is [NPER, A] signs of humaness*action_logits.
    """
    k = len(hidx)
    nper = ns * 8                      # persons incl. padding
    nb = ns * 2                        # batches of 4 persons

    # per-person params, padded
    mu = np.zeros((nper, A, 4), np.float32)
    mu[:k] = target_mean[hidx]
    m2 = (mu * mu).sum(axis=-1)
    lrep = np.zeros((nper, A), np.float32)
    lrep[:k] = best[hidx][:, None] * action_logits[hidx]
    lnl = np.full((nper, A), LNFLOOR * SCALE, np.float32)
    pos = np.abs(lrep) > 0
    lnl[pos] = np.log(np.abs(lrep[pos]))
    lnrow = np.maximum((lnl - m2 * INV2S2) / SCALE, LNFLOOR)   # [nper, A]
    sgn = np.sign(lrep)

    invw = np.ones(nper, np.float32); invw[:k] = 1.0 / w[hidx]
    invh = np.ones(nper, np.float32); invh[:k] = 1.0 / h[hidx]
    cxh = np.zeros(nper, np.float32); cxh[:k] = cx[hidx]
    cyh = np.zeros(nper, np.float32); cyh[:k] = cy[hidx]
    lwh = np.zeros(nper, np.float32); lwh[:k] = np.log(w[hidx])
    lhh = np.zeros(nper, np.float32); lhh[:k] = np.log(h[hidx])

    # lhsT A [nper_group_pairs...]: built per group of 2 persons
    # A rows [KK, 128] per group; B rows [KK, OPC] per (group, core)
    # padded object arrays
    cx_p = np.zeros(NOBJ, np.float32); cx_p[:N] = cx
    cy_p = np.zeros(NOBJ, np.float32); cy_p[:N] = cy
    lw_p = np.zeros(NOBJ, np.float32); lw_p[:N] = np.log(w)
    lh_p = np.zeros(NOBJ, np.float32); lh_p[:N] = np.log(h)

    # t/e2 for all persons x all (padded) objects
    tx = np.clip(cx_p[None, :] * invw[:, None] - (cxh * invw)[:, None],
                 -TCLAMP, TCLAMP)                                  # [nper, NOBJ]
    ty = np.clip(cy_p[None, :] * invh[:, None] - (cyh * invh)[:, None],
                 -TCLAMP, TCLAMP)
    tw = np.clip(lw_p[None, :] - lwh[:, None], -TCLAMP, TCLAMP)
    th = np.clip(lh_p[None, :] - lhh[:, None], -TCLAMP, TCLAMP)
    e2 = tx * tx + ty * ty + tw * tw + th * th

    # A [ngroups, KK, 128], partition q = j*64 + a
    ng = nper // 2
    Af = np.zeros((ng, KK, 2, A), np.float32)
    mug = mu.reshape(ng, 2, A, 4)
    lng = lnrow.reshape(ng, 2, A)
    for j in range(2):
        for c in range(4):
            Af[:, c * 2 + j, j, :] = mug[:, j, :, c]
        Af[:, 8 + j, j, :] = 1.0
        Af[:, 10 + j, j, :] = lng[:, j, :]
    Af[:, 12, :, :] = 1.0
    Af = Af.reshape(ng, KK, 128)
    Ahi, Alo = _hilo(Af)
    A39 = np.concatenate([Ahi, Alo, Ahi], axis=1)      # [ng, KR, 128]

    # B [ngroups, KK, NOBJ]
    Bf = np.zeros((ng, KK, NOBJ), np.float32)
    g2 = lambda x: x.reshape(ng, 2, NOBJ)
    txg, tyg, twg, thg, e2g = g2(tx), g2(ty), g2(tw), g2(th), g2(e2)
    for j in range(2):
        for c, tc in enumerate((txg, tyg, twg, thg)):
            Bf[:, c * 2 + j, :] = tc[:, j, :]
        Bf[:, 8 + j, :] = -0.5 * e2g[:, j, :]
        Bf[:, 10 + j, :] = 1.0
    lnobj_row = np.maximum(lnobj_p / SCALE, LNFLOOR)
    Bf[:, 12, :] = lnobj_row[None, :]
    Bhi, Blo = _hilo(Bf)
    B39 = np.concatenate([Bhi, Bhi, Blo], axis=1)      # [ng, KR, NOBJ]

    in_maps = []
    for c in range(NCORES):
        osl = slice(c * OPC, (c + 1) * OPC)
        blob = np.zeros((ns, KR, SCOLS), np.float16)
        for b in range(nb):
            s, dd = divmod(b, 2)
            for g01 in range(2):
                g = b * 2 + g01
                col0 = dd * BCOLS + g01 * GCOLS
                blob[s, :, col0:col0 + 128] = A39[g]
                blob[s, :, col0 + 128:col0 + 128 + OPC] = B39[g][:, osl]
        in_maps.append({"blob": blob})
    return in_maps, sgn


_NC_CACHE = {}


def _build_nc(ns):
    """Raw-bass program (no TileContext): minimal semaphores, minimal
    preamble/teardown.  Per superstep s (8 persons): one input DMA, four
    matmuls (one per person pair) into separate PSUM banks, one Exp
    activation reading all four banks, one output DMA."""
    if ns in _NC_CACHE:
        return _NC_CACHE[ns]
    import concourse.bacc as bacc
    import concourse.mybir as mybir

    f32 = mybir.dt.float32
    f16 = mybir.dt.float16
    nc = bacc.Bacc()
    blob_d = nc.dram_tensor("blob", [ns, KR, SCOLS], f16, kind="ExternalInput")
    out_d = nc.dram_tensor("out", [ns, 128, 4, OPC], f16, kind="ExternalOutput")

    tins = [nc.alloc_sbuf_tensor(f"tin{s}", [KP, SCOLS], f16) for s in range(ns)]
    ots = [nc.alloc_sbuf_tensor(f"ot{s}", [128, 4, OPC], f16) for s in range(ns)]
    bias = nc.alloc_sbuf_tensor("bias", [128, 1], f32)
    scr = nc.alloc_sbuf_tensor("scr", [128, 1], f16)
    pss = [nc.alloc_psum_tensor(f"ps{i}", [128, 4, 512], f32) for i in range(2)]

    s_ins = [nc.alloc_semaphore(f"s_in{s}") for s in range(ns)]
    s_ms = nc.alloc_semaphore("s_ms")
    s_mm = nc.alloc_semaphore("s_mm")
    s_act = nc.alloc_semaphore("s_act")
    s_out = nc.alloc_semaphore("s_out")
    s_outg = nc.alloc_semaphore("s_outg")

    # Input s -> issuing engine: sync(0), scalar(1), gpsimd(2), then round
    # robin.  All three DMA paths transfer in parallel.
    n_gp_out = sum(1 for s in range(ns) if s % 3 != 0 and s != ns - 1)
    n_out_dma = ns + 1 - n_gp_out   # outs on the two HWDGE rings

    with nc.Block() as block:

        @block.vector
        def _(v):
            v.memset(bias[:, :], 0.0).then_inc(s_ms)             # s_ms: 1
            for s in range(ns):
                v.memset(tins[s][0:PAD, :], 0.0).then_inc(s_ms)  # 2 + s

        @block.sync
        def _(sp):
            for s in range(0, ns, 3):
                sp.dma_start(tins[s][PAD:KP, :], blob_d[s]).then_inc(s_ins[s], 16)
            for s in range(ns):
                if s % 3 != 0 and s != ns - 1:
                    continue
                sp.wait_ge(s_act, s + 1)
                if s == ns - 1:
                    sp.dma_start(out_d[s][:, 0:2], ots[s][:, 0:2]).then_inc(
                        s_out, 16
                    )
                else:
                    sp.dma_start(out_d[s], ots[s][:]).then_inc(s_out, 16)
            sp.wait_ge(s_out, 16 * n_out_dma)
            if n_gp_out:
                sp.wait_ge(s_outg, 16 * n_gp_out)

        @block.scalar
        def _(sc):
            for s in range(1, ns, 3):
                sc.dma_start(tins[s][PAD:KP, :], blob_d[s]).then_inc(s_ins[s], 16)
            # dummy activation: pulls the EXP table load off the critical path
            sc.wait_ge(s_ms, 1)
            sc.activation(
                scr[:, :], bias[:, 0:1],
                mybir.ActivationFunctionType.Exp,
                bias=bias[:, 0:1], scale=1.0,
            )
            for s in range(ns):
                sc.wait_ge(s_mm, 4 * (s + 1))
                sc.activation(
                    ots[s][:], pss[s % 2][:, :, 0:OPC],
                    mybir.ActivationFunctionType.Exp,
                    bias=bias[:, 0:1], scale=float(SCALE),
                ).then_inc(s_act)
            # second half of the last output store, on this ring
            sc.wait_ge(s_act, ns)
            sc.dma_start(
                out_d[ns - 1][:, 2:4], ots[ns - 1][:, 2:4]
            ).then_inc(s_out, 16)

        @block.gpsimd
        def _(gp):
            for s in range(2, ns, 3):
                gp.dma_start(tins[s][PAD:KP, :], blob_d[s]).then_inc(s_ins[s], 16)
            for s in range(ns):
                if s % 3 == 0 or s == ns - 1:
                    continue
                gp.wait_ge(s_act, s + 1)
                gp.dma_start(out_d[s], ots[s][:]).then_inc(s_outg, 16)

        @block.tensor
        def _(te):
            for s in range(ns):
                te.wait_ge(s_ms, s + 2)
                te.wait_ge(s_ins[s], 16)
                if s >= 2:
                    te.wait_ge(s_act, s - 1)   # psum bank reuse (s-2's act)
                ps = pss[s % 2]
                for dd in range(2):
                    for g01 in range(2):
                        col0 = dd * BCOLS + g01 * GCOLS
                        te.matmul(
                            ps[:, 2 * dd + g01, 0:OPC],
                            tins[s][0:KP, col0:col0 + 128],
                            tins[s][0:KP, col0 + 128:col0 + 128 + OPC],
                            start=True, stop=True,
                        ).then_inc(s_mm)

    nc.finalize()
    _NC_CACHE[ns] = nc
    return nc


def _run_sim(in_maps, ns):
    """Numpy emulation of the device program (incl. fp16 rounding)."""
    results = []
    for m in in_maps:
        blob = m["blob"].astype(np.float32)
        out = np.zeros((ns, 128, 4, OPC), np.float32)
        for s in range(ns):
            for dd in range(2):
                for g01 in range(2):
                    col0 = dd * BCOLS + g01 * GCOLS
                    a = blob[s, :, col0:col0 + 128]
                    b = blob[s, :, col0 + 128:col0 + 128 + OPC]
                    mm = a.T @ b
                    out[s, :, 2 * dd + g01, :] = np.exp(
                        np.minimum(SCALE * mm, 80.0))
        results.append({"out": out.astype(np.float16)})
    return results


def _gather(results, ns, k, sgn):
    nper = ns * 8
    parts = []
    for r in results:
        o = np.asarray(r["out"]).astype(np.float32)   # [ns, 128, 4, OPC]
        # partition q = j*64 + a ; bank = 2*dd + g01 ; person = b*4+g01*2+j
        o = o.reshape(ns, 2, A, 2, 2, OPC)            # s, j, a, dd, g01, o
        o = o.transpose(0, 3, 4, 1, 5, 2)             # s, dd, g01, j, o, a
        parts.append(o.reshape(nper, OPC, A))
    full_obj = np.concatenate(parts, axis=1)          # [nper, NOBJ, A]
    return full_obj[:k, :N, :] * sgn[:k, None, :]


def kernel(action_logits, target_mean, bbox, scores):
    action_logits = np.asarray(action_logits, np.float32)
    target_mean = np.asarray(target_mean, np.float32)
    bbox = np.asarray(bbox, np.float32)
    scores = np.asarray(scores, np.float32)

    best = scores.max(axis=1)
    idx = scores.argmax(axis=1)
    person = idx == PERSON_IDX
    obj = np.where(person, 0.0, best).astype(np.float32)

    w = bbox[:, 2] - bbox[:, 0]
    h = bbox[:, 3] - bbox[:, 1]
    cx = bbox[:, 0] + 0.5 * w
    cy = bbox[:, 1] + 0.5 * h

    lnobj_p = np.full(NOBJ, LNFLOOR * SCALE, np.float32)
    pos = obj > 0
    lnobj_p[:N][pos] = np.log(obj[pos])

    hidx = np.where(person)[0]
    k = len(hidx)
    full = np.zeros((N, N, A), np.float32)
    kernel.last_run = None
    if k == 0:
        return full

    ns = max(1, (k + 7) // 8)          # supersteps of 8 persons
    in_maps, sgn = _host_prep(
        hidx, best, w, h, cx, cy, lnobj_p, target_mean, action_logits, ns
    )
    if os.environ.get("KERNEL_SIM") == "1":
        results = _run_sim(in_maps, ns)
    else:
        from concourse.bass_utils import run_bass_kernel_spmd
        nc = _build_nc(ns)
        kw = {}
        if os.environ.get("KERNEL_TRACE") == "1":
            kw = dict(trace=True, trace_cores=list(range(NCORES)))
        r = run_bass_kernel_spmd(
            nc, in_maps, core_ids=list(range(NCORES)), **kw
        )
        results = r.results
        kernel.last_run = r
    full[hidx] = _gather(results, ns, k, sgn)
    return full
